# revision 1
# baseline (speedup 1.0000x reference)
"""Contour -> distance map kernel for 8 Trainium2 NeuronCores.

Math (per polygon p, pixel m, edge k, with vertex v_k and next vertex v_{k+1}):
  diff_k = v_k - m,  roll_k = v_{k+1} - m
  n2_k    = |diff_k|^2
  dot_k   = diff_k . roll_k
  cross_k = diff_k x roll_k
All three are affine in phi(m) = [1, mx, my, mx^2+my^2], so one K=4 matmul
per 128-pixel tile produces (n2 | dot | cross) for all 64 edges.

Reference angle chain  arccos(clip(dot/(nd*nr), -1+eps, 1-eps))  is rewritten
division-light:  theta_k = pi/2 - arctan(clamp(dot/cross, +/-C)),
C = cot(arccos(1-eps)), and
  sum_k tanh(1e5*cross)*theta = (pi/2)*sum(sgn) - sum(|sgn|*arctan(tc))
exactly (signs fold through arctan's oddness).  The DVE min/max clamp
suppresses NaN/Inf from cross==0 reciprocals; those terms are killed by
|sgn|~0 anyway.

Device outputs per core (1 polygon each): SS = sum(sgn), SA = sum(|sgn|*at),
MN = min(n2) as (128, 512) arrays [partition = pixel%128, col = pixel//128].
Host epilogue: wind = |pi/2*SS - SA|/(2pi), prod = wind*sqrt(MN),
dmap = prod / global_max(prod).
"""

import numpy as np

SIZE = 256
EPS = 1e-5
NCORES = 8
K = 64
NTILE = 512          # 128-pixel tiles per core
BATCH = 8            # tiles per batch
NBATCH = NTILE // BATCH
CLAMP = float((1.0 - EPS) / np.sqrt(1.0 - (1.0 - EPS) ** 2))  # 223.607...
MM4 = True           # 4-way matmul concurrency via tile_position row-groups

_CACHE = {}
LAST_RESULTS = None


def _register_custom_ops():
    """Two fused DVE ops (registered into concourse.dve_ops at runtime):

    MULT_CLAMP_ANT: out = max(min(in0*in1, C0), C1)      [3 ALU stages]
        -> tc = clamp(dot * (1/cross), +/-CLAMP).  HW min/max suppress NaN,
        sanitizing 1/0 = NaN/Inf garbage from the reciprocal.
    SGN_THETA_ANT:  out = in0*C0 - |in0|*in1             [5 stages]
        -> c = sgn*(pi/2) - |sgn|*arctan(tc), |x| = max(x, 0-x) on v3.
    """
    import numpy as np
    import concourse.dve_ops as dve_ops
    from concourse.dve_ops import DveOp
    from concourse.dve_spec import Spec, Src0, Src1, C0, C1, Zero, maxx, minn, lower
    from concourse.dve_uop import DveOpSpec

    if "MULT_CLAMP_ANT" in dve_ops._SUB_OPCODE_FOR_NAME:
        return

    def _make(name, spec):
        row = max(dve_ops._SUB_OPCODE_FOR_NAME.values()) + 1
        assert row < 0x20
        dve_ops._SUB_OPCODE_FOR_NAME[name] = row
        shas = {}
        for ver in ("v3", "v4"):
            try:
                uops = lower(spec, ver=ver)
                shas[ver] = DveOpSpec(name=name, opcode=row, uops=uops,
                                      rd1_en=True).sha(ver)
            except Exception:
                pass
        op = DveOp(name, spec, subdim=False, uops_sha=shas)
        dve_ops.OPS.append(op)
        dve_ops.CUSTOM_DVE_SPECS[name] = spec
        return op

    mc = _make(
        "MULT_CLAMP_ANT",
        Spec(
            body=maxx(minn(Src0 * Src1, C0), C1),
            reference=lambda in0, in1, c0, c1, c2: np.maximum(
                np.minimum(in0 * in1, c0), c1),
        ),
    )
    st = _make(
        "SGN_THETA_ANT",
        Spec(
            body=Src0 * C0 - maxx(Src0, Zero - Src0) * Src1,
            reference=lambda in0, in1, c0, c1, c2: in0 * c0
            - np.abs(in0) * in1,
        ),
    )
    _CACHE["ops"] = (mc, st)


def _patch_act_tables():
    """Keep Tanh/Arctan only in sigmoid_and_others so the bacc table-load
    pass resolves both to ONE set (otherwise Tanh->exp_and_others and
    Arctan->sigmoid_and_others thrash the ~2.7us ACT table load per
    transition). Set ids/order are untouched - only membership is edited."""
    import concourse.bacc as bacc
    from concourse import mybir
    if getattr(bacc, "_ant_act_tables_patched", False):
        return
    orig = bacc.get_activation_tables

    def patched(module_arch):
        tables = orig(module_arch)
        keep = "sigmoid_and_others"
        for name, funcs in tables.items():
            if name == keep:
                continue
            funcs.discard(mybir.ActivationFunctionType.Tanh)
            funcs.discard(mybir.ActivationFunctionType.Arctan)
        return tables

    bacc.get_activation_tables = patched
    bacc._ant_act_tables_patched = True


def _build_program(reps=1, skip=()):
    import concourse.bacc as bacc
    import concourse.tile as tile
    from concourse import mybir
    import concourse.bass as bass

    _register_custom_ops()
    _patch_act_tables()
    mc_op, st_op = _CACHE["ops"]
    skip = set(skip)

    f32 = mybir.dt.float32
    AF = mybir.ActivationFunctionType
    ALU = mybir.AluOpType

    nc = bacc.Bacc("TRN2", target_bir_lowering=False, debug=False,
                   num_devices=NCORES)

    if MM4:
        # phi rows replicated at 4 partition offsets (32g..32g+4); tile T
        # lives at row-group g = (T%8)//2, block column 2*(T//8) + T%2.
        phiT = nc.dram_tensor("phiT", [128, NTILE // 4, 128], f32,
                              kind="ExternalInput")
        wmat = nc.dram_tensor("wmat", [128, 3 * K], f32,
                              kind="ExternalInput")
    else:
        phiT = nc.dram_tensor("phiT", [4, NTILE, 128], f32,
                              kind="ExternalInput")
        wmat = nc.dram_tensor("wmat", [4, 3 * K], f32, kind="ExternalInput")
    sc_d = nc.dram_tensor("sc", [128, NTILE], f32, kind="ExternalOutput")
    mn_d = nc.dram_tensor("mn", [128, NTILE], f32, kind="ExternalOutput")

    import os
    CHUNK = 64   # tiles of phiT per staged DMA chunk
    SUPER = int(os.environ.get("ANT_SUPER", "2"))
    SB_T = SUPER * BATCH  # tiles per super-batch
    WBUFS = int(os.environ.get("ANT_WBUFS", "2"))
    KBUFS = int(os.environ.get("ANT_KBUFS", "3"))

    with tile.TileContext(nc) as tc:
        with (
            tc.tile_pool(name="wpool", bufs=1) as wpool,
            tc.tile_pool(name="chunkpool", bufs=2) as chunkpool,
            tc.tile_pool(name="psum", bufs=2, space="PSUM") as psum_pool,
            tc.tile_pool(name="work", bufs=KBUFS) as work,
            tc.tile_pool(name="wide", bufs=WBUFS) as wide,
            tc.tile_pool(name="outs", bufs=1) as outs,
        ):
            w_s = wpool.tile([128, 3 * K] if MM4 else [4, 3 * K], f32)
            nc.sync.dma_start(w_s[:], wmat[:])

            sc_t = outs.tile([128, NTILE], f32)
            mn_t = outs.tile([128, NTILE], f32)

            def body():
                chunk = None
                for sb in range(NBATCH // SUPER):
                    sg_w = wide.tile([128, SB_T, K], f32, tag="sg")
                    tc_w = wide.tile([128, SB_T, K], f32, tag="tc")
                    for j in range(SUPER):
                        b = sb * SUPER + j
                        if b % (CHUNK // BATCH) == 0:
                            c = b // (CHUNK // BATCH)
                            if MM4:
                                chunk = chunkpool.tile(
                                    [128, CHUNK // 4, 128], f32, tag="chunk")
                                nc.sync.dma_start(
                                    chunk[:],
                                    phiT[:, c * (CHUNK // 4):
                                         (c + 1) * (CHUNK // 4), :])
                            else:
                                chunk = chunkpool.tile([4, CHUNK, 128], f32,
                                                       tag="chunk")
                                nc.sync.dma_start(
                                    chunk[:],
                                    phiT[:, c * CHUNK:(c + 1) * CHUNK, :])
                        j0 = (b % (CHUNK // BATCH)) * BATCH

                        pt = psum_pool.tile([128, BATCH, 256], f32, tag="pt")
                        if MM4:
                            # quads {0,2,4,6} then {1,3,5,7}: 4 concurrent
                            # MMs on distinct row-groups -> distinct banks.
                            for t in [0, 2, 4, 6, 1, 3, 5, 7]:
                                T = b * BATCH + t
                                g = t // 2
                                lblk = 2 * ((T // 8) % (CHUNK // 8)) + T % 2
                                nc.tensor.matmul(
                                    pt[:, t, 0:3 * K],
                                    chunk[32 * g:32 * g + 4, lblk, :],
                                    w_s[32 * g:32 * g + 4, :],
                                    start=True, stop=True,
                                    tile_position=(32 * g, 0),
                                )
                        else:
                            for t in range(BATCH):
                                nc.tensor.matmul(
                                    pt[:, t, 0:3 * K],
                                    chunk[:, j0 + t, :],
                                    w_s[:],
                                    start=True, stop=True,
                                )
                        n2 = pt[:, :, 0:K]
                        dot = pt[:, :, K:2 * K]
                        cross = pt[:, :, 2 * K:3 * K]
                        jsl = slice(j * BATCH, (j + 1) * BATCH)

                        if "tanh" not in skip:
                            nc.scalar.activation(sg_w[:, jsl, :], cross,
                                                 AF.Tanh, scale=100000.0)
                        if "rc" not in skip:
                            rc = work.tile([128, BATCH, K], f32, tag="rc")
                            nc.vector.reciprocal_approx_fast(rc[:], cross)
                            nc.vector._custom_dve(
                                mc_op, out=tc_w[:, jsl, :], in0=dot,
                                in1=rc[:], s0=CLAMP, s1=-CLAMP)
                        if "min" not in skip:
                            nc.vector.tensor_reduce(
                                mn_t[:, bass.ts(b, BATCH)], n2,
                                axis=mybir.AxisListType.X, op=ALU.min)

                    at_w = wide.tile([128, SB_T, K], f32, tag="at")
                    if "at" not in skip:
                        nc.scalar.activation(at_w[:], tc_w[:], AF.Arctan)
                    if "c" not in skip:
                        c_w = wide.tile([128, SB_T, K], f32, tag="c")
                        nc.vector._custom_dve(
                            st_op, out=c_w[:], in0=sg_w[:], in1=at_w[:],
                            s0=float(np.pi / 2), s1=0.0)
                        nc.vector.tensor_reduce(
                            sc_t[:, bass.ts(sb, SB_T)], c_w[:],
                            axis=mybir.AxisListType.X, op=ALU.add)

            if reps > 1:
                with tc.For_i(0, reps, 1,
                              hint_engines=(mybir.EngineType.PE,
                                            mybir.EngineType.DVE)):
                    body()
            else:
                body()

            nc.sync.dma_start(sc_d[:], sc_t[:])
            nc.sync.dma_start(mn_d[:], mn_t[:])

    nc.compile()
    return nc


def _host_inputs(contour):
    """Per-core input maps: shared phi(mesh) lhsT and per-polygon W."""
    C = contour.reshape(NCORES, K, 2).astype(np.float64)
    cx, cy = C[..., 0], C[..., 1]
    cxn, cyn = np.roll(cx, -1, 1), np.roll(cy, -1, 1)
    ones = np.ones_like(cx)
    Wn2 = np.stack([cx * cx + cy * cy, -2 * cx, -2 * cy, ones], 1)
    Wdot = np.stack([cx * cxn + cy * cyn, -(cx + cxn), -(cy + cyn), ones], 1)
    Wcr = np.stack([cy * cxn - cx * cyn, cyn - cy, cx - cxn,
                    np.zeros_like(cx)], 1)
    W = np.concatenate([Wn2, Wdot, Wcr], axis=2).astype(np.float32)  # (8,4,192)

    ax = np.arange(SIZE) / SIZE
    gx, gy = np.meshgrid(ax, ax, indexing="ij")
    mx, my = gx.ravel(), gy.ravel()
    phi = np.stack([np.ones(SIZE * SIZE), mx, my, mx * mx + my * my], 0)
    phi = phi.astype(np.float32)

    if MM4:
        phi4 = np.zeros((128, NTILE // 4, 128), np.float32)
        for T in range(NTILE):
            g = (T % 8) // 2
            blk = 2 * (T // 8) + T % 2
            phi4[32 * g:32 * g + 4, blk, :] = phi[:, 128 * T:128 * (T + 1)]
        w4 = np.zeros((NCORES, 128, 3 * K), np.float32)
        for g in range(4):
            w4[:, 32 * g:32 * g + 4, :] = W
        return [{"phiT": phi4, "wmat": np.ascontiguousarray(w4[p])}
                for p in range(NCORES)]

    phiT = phi.reshape(4, NTILE, 128)
    return [{"phiT": phiT, "wmat": np.ascontiguousarray(W[p])}
            for p in range(NCORES)]


def _get_executor(reps=1, skip=()):
    """Build (once) a reusable jitted SPMD executor over the 8 cores.

    Mirrors concourse.bass2jax.run_bass_via_pjrt but without output-buffer
    donation: every output element is fully written by the kernel, so the
    zero output operands can be uploaded once and reused across calls.
    """
    key = ("exec", reps, tuple(sorted(skip)))
    if key in _CACHE:
        return _CACHE[key]

    import jax
    import jax.numpy as jnp  # noqa: F401
    from jax.sharding import Mesh, PartitionSpec, NamedSharding
    from jax.experimental.shard_map import shard_map
    import concourse.mybir as mybir
    from concourse.bass2jax import _bass_exec_p, install_neuronx_cc_hook

    install_neuronx_cc_hook()
    nckey = ("nc", reps, tuple(sorted(skip)))
    if nckey not in _CACHE:
        _CACHE[nckey] = _build_program(reps=reps, skip=skip)
    nc = _CACHE[nckey]
    partition_name = (nc.partition_id_tensor.name
                      if nc.partition_id_tensor else None)

    in_names, out_names, out_avals, zero_outs = [], [], [], []
    for alloc in nc.m.functions[0].allocations:
        if not isinstance(alloc, mybir.MemoryLocationSet):
            continue
        name = alloc.memorylocations[0].name
        if alloc.kind == "ExternalInput":
            if name == partition_name:
                continue
            in_names.append(name)
        elif alloc.kind == "ExternalOutput":
            out_names.append(name)
            shape = tuple(alloc.tensor_shape)
            dtype = mybir.dt.np(alloc.dtype)
            out_avals.append(jax.core.ShapedArray(shape, dtype))
            zero_outs.append(np.zeros(shape, dtype))
    n_params = len(in_names)
    all_names = in_names + out_names
    if partition_name is not None:
        all_names = all_names + [partition_name]

    from concourse.bass2jax import partition_id_tensor

    def _body(*args):
        operands = list(args)
        if partition_name is not None:
            operands.append(partition_id_tensor())
        outs = _bass_exec_p.bind(
            *operands,
            out_avals=tuple(out_avals),
            in_names=tuple(all_names),
            out_names=tuple(out_names),
            lowering_input_output_aliases=(),
            sim_require_finite=True,
            sim_require_nnan=True,
            nc=nc,
        )
        return tuple(outs)

    devices = jax.devices()[:NCORES]
    mesh = Mesh(np.asarray(devices), ("core",))
    nspec = (PartitionSpec("core"),) * (n_params + len(out_names))
    sharded = jax.jit(
        shard_map(_body, mesh=mesh, in_specs=nspec,
                  out_specs=(PartitionSpec("core"),) * len(out_names),
                  check_rep=False),
        keep_unused=True,
    )
    sharding = NamedSharding(mesh, PartitionSpec("core"))
    zeros_dev = [
        jax.device_put(
            np.zeros((NCORES * z.shape[0], *z.shape[1:]), z.dtype), sharding)
        for z in zero_outs
    ]
    _CACHE[key] = (sharded, sharding, in_names, out_names, zeros_dev)
    return _CACHE[key]


def _run(contour):
    """Returns list (per core) of dicts {sc, mn} as np arrays."""
    import jax
    sharded, sharding, in_names, out_names, zeros_dev = _get_executor()
    in_maps = _host_inputs(contour)
    concat = {
        name: np.concatenate([m[name] for m in in_maps], axis=0)
        for name in in_names
    }
    if "phiT_dev" not in _CACHE:
        _CACHE["phiT_dev"] = jax.device_put(concat["phiT"], sharding)
    ins = [
        _CACHE["phiT_dev"] if name == "phiT"
        else jax.device_put(concat[name], sharding)
        for name in in_names
    ]
    outs = sharded(*ins, *zeros_dev)
    res = []
    per_core_rows = {n: concat[n].shape[0] // NCORES for n in in_names}
    del per_core_rows
    for c in range(NCORES):
        d = {}
        for i, name in enumerate(out_names):
            arr = np.asarray(outs[i])
            rows = arr.shape[0] // NCORES
            d[name] = arr[c * rows:(c + 1) * rows]
        res.append(d)
    return res


def benchmark(contour, iters=20, reps=1, skip=()):
    """Pipelined repeated execution; returns avg seconds/iteration."""
    import time
    import jax
    sharded, sharding, in_names, out_names, zeros_dev = _get_executor(
        reps, skip)
    in_maps = _host_inputs(np.asarray(contour, dtype=np.float32))
    concat = {
        name: np.concatenate([m[name] for m in in_maps], axis=0)
        for name in in_names
    }
    ins = [jax.device_put(concat[name], sharding) for name in in_names]
    out = sharded(*ins, *zeros_dev)  # warm-up
    jax.block_until_ready(out)
    t0 = time.time()
    outs = [sharded(*ins, *zeros_dev) for _ in range(iters)]
    jax.block_until_ready(outs[-1])
    t1 = time.time()
    return (t1 - t0) / iters


def kernel(contour, *, _trace=False):
    contour = np.asarray(contour, dtype=np.float32)
    results = _run(contour)

    planes = []
    for p in range(NCORES):
        out = results[p]
        S = out["sc"].T.ravel()
        mn = out["mn"].T.ravel()
        wind = np.abs(S * np.float32(1.0 / (2 * np.pi)))
        dist = np.sqrt(np.maximum(mn, np.float32(0)))
        planes.append((wind * dist).astype(np.float32))
    prod = np.stack(planes)                      # (8, 65536)
    dmap = (prod / prod.max()).astype(np.float32)
    return dmap.reshape(2, 4, SIZE, SIZE)



# revision 3
# speedup vs baseline: 1.1171x; 1.1171x over previous
"""Contour -> distance map kernel for 8 Trainium2 NeuronCores.

Math (per polygon, pixel m=(mx,my), edge k: vertex a=v_k, next b=v_{k+1}):
  cross_k = (a-m) x (b-m)   (affine in [1, mx, my])
  n2_k    = |a-m|^2         (affine in phi(m) = [1, mx, my, mx^2+my^2])
The reference's tanh/arccos winding-angle sum equals (a.e.) the integer
winding number, which the kernel computes by signed ray-crossing counting:
  W = 1/2 * sum_k tanh(1e5*cross_k) * h_k,
  h_k = [sgn(vy_k - my) != sgn(vy_{k+1} - my)]  (edge spans the pixel row).
h depends only on (edge, my); my has a fixed 128-lane pattern per pixel-tile
parity, so h is a host-precomputed constant SBUF table - no dot products,
reciprocals or arctans on device (CPU-checked rel err 2.7e-4 vs reference).

Sharding: core i handles polygon pair {2*(i//2), 2*(i//2)+1} on pixel half
i%2 (32768 px).  Per 128-pixel tile ONE fp32r matmul (1 cycle/col at >=256
out cols) yields psum[128, (poly, kind, 64)] = n2|cross for both polygons.
Then: ACT tanh(1e5*cross)->bf16, DVE c = t1*h (bf16 2x mode), DVE add-reduce
c -> winding sum, DVE min-reduce n2.  Host epilogue: wind = |SC|/2,
prod = wind*sqrt(MN), dmap = prod / global_max(prod).
"""

import numpy as np

SIZE = 256
NCORES = 8
K = 64
NPOLY = 2            # polygons per core
NT = 256             # 128-pixel tiles per core (half the image)
BATCH = 8            # tiles per psum batch
NBATCH = NT // BATCH

_CACHE = {}


def _build_program(reps=1, skip=()):
    import concourse.bacc as bacc
    import concourse.tile as tile
    from concourse import mybir
    import concourse.bass as bass

    skip = set(skip)
    f32 = mybir.dt.float32
    f32r = mybir.dt.float32r
    bf16 = mybir.dt.bfloat16
    AF = mybir.ActivationFunctionType
    ALU = mybir.AluOpType

    nc = bacc.Bacc("TRN2", target_bir_lowering=False, debug=False,
                   num_devices=NCORES)

    phi_d = nc.dram_tensor("phi", [4, NT, 128], f32, kind="ExternalInput")
    w_d = nc.dram_tensor("wmat", [4, NPOLY * 2 * K], f32,
                         kind="ExternalInput")
    h_d = nc.dram_tensor("hrep", [128, BATCH, NPOLY, K], bf16,
                         kind="ExternalInput")
    sc_d = nc.dram_tensor("sc", [128, NT * NPOLY], f32, kind="ExternalOutput")
    mn_d = nc.dram_tensor("mn", [128, NT * NPOLY], f32, kind="ExternalOutput")

    import os
    KBUFS = int(os.environ.get("ANT_KBUFS", "3"))

    with tile.TileContext(nc) as tc:
        with (
            tc.tile_pool(name="const", bufs=1) as cpool,
            tc.tile_pool(name="psum", bufs=2, space="PSUM") as psum_pool,
            tc.tile_pool(name="work", bufs=KBUFS) as work,
            tc.tile_pool(name="outs", bufs=1) as outs,
        ):
            phi_s = cpool.tile([4, NT, 128], f32)
            w_s = cpool.tile([4, NPOLY * 2 * K], f32)
            h_s = cpool.tile([128, BATCH, NPOLY, K], bf16)
            nc.sync.dma_start(phi_s[:], phi_d[:])
            nc.sync.dma_start(w_s[:], w_d[:])
            nc.sync.dma_start(h_s[:], h_d[:])

            sc_t = outs.tile([128, NT * NPOLY], f32)
            mn_t = outs.tile([128, NT * NPOLY], f32)

            def body():
                for b in range(NBATCH):
                    pt = psum_pool.tile([128, BATCH, NPOLY, 2, K], f32,
                                        tag="pt")
                    for t in range(BATCH):
                        T = b * BATCH + t
                        nc.tensor.matmul(
                            pt[:, t, :, :, :],
                            phi_s[:, T, :],
                            w_s[:],
                            start=True, stop=True,
                        )
                    n2 = pt[:, :, :, 0, :]
                    cross = pt[:, :, :, 1, :]
                    osl = slice(b * BATCH * NPOLY, (b + 1) * BATCH * NPOLY)

                    if "min" not in skip:
                        nc.vector.tensor_reduce(
                            mn_t[:, osl], n2,
                            axis=mybir.AxisListType.X, op=ALU.min)
                    if "tanh" not in skip:
                        t1 = work.tile([128, BATCH, NPOLY, K], bf16, tag="t1")
                        nc.scalar.activation(t1[:], cross, AF.Tanh,
                                             scale=100000.0)
                    if "mult" not in skip:
                        c_w = work.tile([128, BATCH, NPOLY, K], bf16, tag="c")
                        nc.vector.tensor_tensor(c_w[:], t1[:], h_s[:],
                                                op=ALU.mult)
                    if "add" not in skip:
                        nc.vector.tensor_reduce(
                            sc_t[:, osl], c_w[:],
                            axis=mybir.AxisListType.X, op=ALU.add)

            if reps > 1:
                with tc.For_i(0, reps, 1,
                              hint_engines=(mybir.EngineType.PE,
                                            mybir.EngineType.DVE)):
                    body()
            else:
                body()

            nc.sync.dma_start(sc_d[:], sc_t[:])
            nc.sync.dma_start(mn_d[:], mn_t[:])

    nc.compile()
    return nc


def _host_inputs(contour):
    """Per-core input maps: phi (pixel half), W + h tables (polygon pair)."""
    import ml_dtypes
    C = contour.reshape(NCORES, K, 2).astype(np.float64)

    ax = np.arange(SIZE) / SIZE
    m = np.arange(SIZE * SIZE)
    mx = (m // SIZE) / SIZE
    my = (m % SIZE) / SIZE
    phi_full = np.stack([np.ones_like(mx), mx, my, mx * mx + my * my], 0)

    maps = []
    for i in range(NCORES):
        a = i // 2
        half = i % 2
        polys = [2 * a, 2 * a + 1]

        wmat = np.zeros((4, NPOLY * 2 * K))
        hrep = np.zeros((128, BATCH, NPOLY, K))
        for pi, p in enumerate(polys):
            vx, vy = C[p, :, 0], C[p, :, 1]
            vxn, vyn = np.roll(vx, -1), np.roll(vy, -1)
            base = pi * 2 * K
            wmat[:, base:base + K] = np.stack(
                [vx * vx + vy * vy, -2 * vx, -2 * vy, np.ones(K)], 0)
            wmat[:, base + K:base + 2 * K] = np.stack(
                [vy * vxn - vx * vyn, vyn - vy, vx - vxn, np.zeros(K)], 0)
            # h table over all 256 my values
            dy = vy[None, :] - ax[:, None]      # (256, K)
            dyn = vyn[None, :] - ax[:, None]
            H = (np.sign(dy) != np.sign(dyn)).astype(np.float64)
            for t in range(BATCH):
                hrep[:, t, pi, :] = H[(t % 2) * 128:(t % 2) * 128 + 128, :]

        sl = slice(half * 32768, (half + 1) * 32768)
        phi = phi_full[:, sl].reshape(4, NT, 128)
        maps.append({
            "phi": np.ascontiguousarray(phi, dtype=np.float32),
            "wmat": wmat.astype(np.float32),
            "hrep": hrep.astype(ml_dtypes.bfloat16),
        })
    return maps


def _get_executor(reps=1, skip=()):
    """Build (once) a reusable jitted SPMD executor over the 8 cores."""
    key = ("exec", reps, tuple(sorted(skip)))
    if key in _CACHE:
        return _CACHE[key]

    import jax
    from jax.sharding import Mesh, PartitionSpec, NamedSharding
    from jax.experimental.shard_map import shard_map
    import concourse.mybir as mybir
    from concourse.bass2jax import _bass_exec_p, install_neuronx_cc_hook

    install_neuronx_cc_hook()
    nckey = ("nc", reps, tuple(sorted(skip)))
    if nckey not in _CACHE:
        _CACHE[nckey] = _build_program(reps=reps, skip=skip)
    nc = _CACHE[nckey]
    partition_name = (nc.partition_id_tensor.name
                      if nc.partition_id_tensor else None)

    in_names, out_names, out_avals, zero_outs = [], [], [], []
    for alloc in nc.m.functions[0].allocations:
        if not isinstance(alloc, mybir.MemoryLocationSet):
            continue
        name = alloc.memorylocations[0].name
        if alloc.kind == "ExternalInput":
            if name == partition_name:
                continue
            in_names.append(name)
        elif alloc.kind == "ExternalOutput":
            out_names.append(name)
            shape = tuple(alloc.tensor_shape)
            dtype = mybir.dt.np(alloc.dtype)
            out_avals.append(jax.core.ShapedArray(shape, dtype))
            zero_outs.append(np.zeros(shape, dtype))
    n_params = len(in_names)
    all_names = in_names + out_names
    if partition_name is not None:
        all_names = all_names + [partition_name]

    from concourse.bass2jax import partition_id_tensor

    def _body(*args):
        operands = list(args)
        if partition_name is not None:
            operands.append(partition_id_tensor())
        outs = _bass_exec_p.bind(
            *operands,
            out_avals=tuple(out_avals),
            in_names=tuple(all_names),
            out_names=tuple(out_names),
            lowering_input_output_aliases=(),
            sim_require_finite=True,
            sim_require_nnan=True,
            nc=nc,
        )
        return tuple(outs)

    devices = jax.devices()[:NCORES]
    mesh = Mesh(np.asarray(devices), ("core",))
    nspec = (PartitionSpec("core"),) * (n_params + len(out_names))
    sharded = jax.jit(
        shard_map(_body, mesh=mesh, in_specs=nspec,
                  out_specs=(PartitionSpec("core"),) * len(out_names),
                  check_rep=False),
        keep_unused=True,
    )
    sharding = NamedSharding(mesh, PartitionSpec("core"))
    zeros_dev = [
        jax.device_put(
            np.zeros((NCORES * z.shape[0], *z.shape[1:]), z.dtype), sharding)
        for z in zero_outs
    ]
    _CACHE[key] = (sharded, sharding, in_names, out_names, zeros_dev)
    return _CACHE[key]


def _run(contour):
    """Returns list (per core) of dicts {sc, mn} as np arrays."""
    import jax
    sharded, sharding, in_names, out_names, zeros_dev = _get_executor()
    in_maps = _host_inputs(contour)
    concat = {
        name: np.concatenate([m[name] for m in in_maps], axis=0)
        for name in in_names
    }
    if "phi_dev" not in _CACHE:
        _CACHE["phi_dev"] = jax.device_put(concat["phi"], sharding)
    ins = [
        _CACHE["phi_dev"] if name == "phi"
        else jax.device_put(concat[name], sharding)
        for name in in_names
    ]
    outs = sharded(*ins, *zeros_dev)
    res = []
    for c in range(NCORES):
        d = {}
        for i, name in enumerate(out_names):
            arr = np.asarray(outs[i])
            rows = arr.shape[0] // NCORES
            d[name] = arr[c * rows:(c + 1) * rows]
        res.append(d)
    return res


def benchmark(contour, iters=20, reps=1, skip=()):
    """Pipelined repeated execution; returns avg seconds/iteration."""
    import time
    import jax
    sharded, sharding, in_names, out_names, zeros_dev = _get_executor(
        reps, skip)
    in_maps = _host_inputs(np.asarray(contour, dtype=np.float32))
    concat = {
        name: np.concatenate([m[name] for m in in_maps], axis=0)
        for name in in_names
    }
    ins = [jax.device_put(concat[name], sharding) for name in in_names]
    out = sharded(*ins, *zeros_dev)  # warm-up
    jax.block_until_ready(out)
    t0 = time.time()
    outs = [sharded(*ins, *zeros_dev) for _ in range(iters)]
    jax.block_until_ready(outs[-1])
    t1 = time.time()
    return (t1 - t0) / iters


def kernel(contour, *, _trace=False):
    contour = np.asarray(contour, dtype=np.float32)
    results = _run(contour)

    prod = np.zeros((NCORES, SIZE * SIZE), np.float32)
    for i in range(NCORES):
        a = i // 2
        half = i % 2
        sl = slice(half * 32768, (half + 1) * 32768)
        S = results[i]["sc"].reshape(128, NT, NPOLY)
        M = results[i]["mn"].reshape(128, NT, NPOLY)
        for pi in range(NPOLY):
            wind = np.abs(S[:, :, pi].T.ravel()) * np.float32(0.5)
            dist = np.sqrt(np.maximum(M[:, :, pi].T.ravel(), 0.0))
            prod[2 * a + pi, sl] = wind * dist
    dmap = (prod / prod.max()).astype(np.float32)
    return dmap.reshape(2, 4, SIZE, SIZE)


# revision 8
# speedup vs baseline: 1.5823x; 1.4165x over previous
"""Contour -> distance map kernel for 8 Trainium2 NeuronCores.

Math (per polygon, pixel m=(mx,my), edge k: vertex a=v_k, next b=v_{k+1}):
  cross_k = (a-m) x (b-m)   (affine in [1, mx, my])
  n2_k    = |a-m|^2         (affine in phi(m) = [1, mx, my, mx^2+my^2])
The reference's tanh/arccos winding-angle sum equals (a.e.) the integer
winding number, which the kernel computes by signed ray-crossing counting:
  W = 1/2 * sum_k tanh(1e5*cross_k) * h_k,
  h_k = [sgn(vy_k - my) != sgn(vy_{k+1} - my)]  (edge spans the pixel row).
h depends only on (edge, my); my has a fixed 128-lane pattern per pixel-tile
parity, so h is a host-precomputed constant SBUF table - no dot products,
reciprocals or arctans on device (CPU-checked rel err 2.7e-4 vs reference).

Sharding: core i handles polygon pair {2*(i//2), 2*(i//2)+1} on pixel half
i%2 (32768 px).  Per 128-pixel tile ONE fp32r matmul (1 cycle/col at >=256
out cols) yields psum[128, (poly, kind, 64)] = n2|cross for both polygons.
Then: ACT tanh(1e5*cross)->bf16, DVE c = t1*h (bf16 2x mode), DVE add-reduce
c -> winding sum, DVE min-reduce n2.  Host epilogue: wind = |SC|/2,
prod = wind*sqrt(MN), dmap = prod / global_max(prod).
"""

import numpy as np

SIZE = 256
NCORES = 8
K = 64
NPOLY = 2            # polygons per core
NT = 256             # 128-pixel tiles per core (half the image)
BATCH = 8            # tiles per psum batch
NBATCH = NT // BATCH

_CACHE = {}


def _build_program(reps=1, skip=()):
    import concourse.bacc as bacc
    import concourse.tile as tile
    from concourse import mybir
    import concourse.bass as bass

    skip = set(skip)
    f32 = mybir.dt.float32
    f32r = mybir.dt.float32r
    bf16 = mybir.dt.bfloat16
    AF = mybir.ActivationFunctionType
    ALU = mybir.AluOpType

    nc = bacc.Bacc("TRN2", target_bir_lowering=False, debug=False,
                   num_devices=NCORES)

    phi_d = nc.dram_tensor("phi", [24, NT, 128], bf16, kind="ExternalInput")
    w_d = nc.dram_tensor("wmat", [24, NPOLY * 2 * K], bf16,
                         kind="ExternalInput")
    h_d = nc.dram_tensor("hrep", [128, BATCH, NPOLY, K], bf16,
                         kind="ExternalInput")
    sc_d = nc.dram_tensor("sc", [128, NT * NPOLY], f32, kind="ExternalOutput")
    mn_d = nc.dram_tensor("mn", [128, NT * NPOLY], f32, kind="ExternalOutput")

    import os
    KBUFS = int(os.environ.get("ANT_KBUFS", "3"))

    with tile.TileContext(nc) as tc:
        with (
            tc.tile_pool(name="const", bufs=1) as cpool,
            tc.tile_pool(name="psum", bufs=2, space="PSUM") as psum_pool,
            tc.tile_pool(name="work", bufs=KBUFS) as work,
            tc.tile_pool(name="outs", bufs=1) as outs,
        ):
            phi_s = cpool.tile([24, NT, 128], bf16)
            w_s = cpool.tile([24, NPOLY * 2 * K], bf16)
            h_s = cpool.tile([128, BATCH, NPOLY, K], bf16)
            nc.sync.dma_start(phi_s[:], phi_d[:])
            nc.sync.dma_start(w_s[:], w_d[:])
            nc.sync.dma_start(h_s[:], h_d[:])

            sc_t = outs.tile([128, NT * NPOLY], f32)
            mn_t = outs.tile([128, NT * NPOLY], f32)

            def body():
                for b in range(NBATCH):
                    pt = psum_pool.tile([128, BATCH, NPOLY, 2, K], f32,
                                        tag="pt")
                    for t in range(BATCH):
                        T = b * BATCH + t
                        nc.tensor.matmul(
                            pt[:, t, :, :, :],
                            phi_s[:, T, :],
                            w_s[:],
                            start=True, stop=True,
                        )
                    n2 = pt[:, :, :, 0, :]
                    cross = pt[:, :, :, 1, :]
                    osl = slice(b * BATCH * NPOLY, (b + 1) * BATCH * NPOLY)

                    if "min" not in skip:
                        nc.vector.tensor_reduce(
                            mn_t[:, osl], n2,
                            axis=mybir.AxisListType.X, op=ALU.min)
                    if "tanh" not in skip:
                        t1 = work.tile([128, BATCH, NPOLY, K], bf16, tag="t1")
                        nc.scalar.activation(t1[:], cross, AF.Tanh,
                                             scale=100000.0)
                    if "mult" not in skip:
                        c_w = work.tile([128, BATCH, NPOLY, K], bf16, tag="c")
                        nc.vector.tensor_tensor(c_w[:], t1[:], h_s[:],
                                                op=ALU.mult)
                    if "add" not in skip:
                        nc.vector.tensor_reduce(
                            sc_t[:, osl], c_w[:],
                            axis=mybir.AxisListType.X, op=ALU.add)

            if reps > 1:
                with tc.For_i(0, reps, 1,
                              hint_engines=(mybir.EngineType.PE,
                                            mybir.EngineType.DVE)):
                    body()
            else:
                body()

            nc.sync.dma_start(sc_d[:], sc_t[:])
            nc.sync.dma_start(mn_d[:], mn_t[:])

    nc.compile()
    return nc


def _host_inputs(contour):
    """Per-core input maps: phi (pixel half), W + h tables (polygon pair)."""
    import ml_dtypes
    C = contour.reshape(NCORES, K, 2).astype(np.float64)

    ax = np.arange(SIZE) / SIZE
    m = np.arange(SIZE * SIZE)
    mx = (m // SIZE) / SIZE
    my = (m % SIZE) / SIZE
    phi_full = np.stack([np.ones_like(mx), mx, my, mx * mx + my * my], 0)

    maps = []
    for i in range(NCORES):
        a = i // 2
        half = i % 2
        polys = [2 * a, 2 * a + 1]

        wmat = np.zeros((4, NPOLY * 2 * K))
        hrep = np.zeros((128, BATCH, NPOLY, K))
        for pi, p in enumerate(polys):
            vx, vy = C[p, :, 0], C[p, :, 1]
            vxn, vyn = np.roll(vx, -1), np.roll(vy, -1)
            base = pi * 2 * K
            wmat[:, base:base + K] = np.stack(
                [vx * vx + vy * vy, -2 * vx, -2 * vy, np.ones(K)], 0)
            wmat[:, base + K:base + 2 * K] = np.stack(
                [vy * vxn - vx * vyn, vyn - vy, vx - vxn, np.zeros(K)], 0)
            # h table over all 256 my values
            dy = vy[None, :] - ax[:, None]      # (256, K)
            dyn = vyn[None, :] - ax[:, None]
            H = (np.sign(dy) != np.sign(dyn)).astype(np.float64)
            for t in range(BATCH):
                hrep[:, t, pi, :] = H[(t % 2) * 128:(t % 2) * 128 + 128, :]

        wmat = wmat.astype(np.float32).astype(np.float64)
        sl = slice(half * 32768, (half + 1) * 32768)
        phi = phi_full[:, sl].astype(np.float32).astype(np.float64)

        # Staggered bf16 splits: (phi_h+phi_m+phi_l)(w_h+w_m+w_l) expanded,
        # keeping the 6 product pairs >= 2^-24: one K=24 bf16 matmul whose
        # fp32-psum result matches the fp32 matmul to ~1.5e-7.
        def split3(x):
            h = x.astype(ml_dtypes.bfloat16).astype(np.float64)
            r = x - h
            mi = r.astype(ml_dtypes.bfloat16).astype(np.float64)
            lo = (r - mi).astype(ml_dtypes.bfloat16).astype(np.float64)
            return h, mi, lo

        ph, pm, pl = split3(phi)
        wh, wm, wl = split3(wmat)
        phi24 = np.concatenate([ph, ph, pm, ph, pm, pl], 0).reshape(
            24, NT, 128)
        w24 = np.concatenate([wh, wm, wh, wl, wm, wh], 0)
        maps.append({
            "phi": phi24.astype(ml_dtypes.bfloat16),
            "wmat": w24.astype(ml_dtypes.bfloat16),
            "hrep": hrep.astype(ml_dtypes.bfloat16),
        })
    return maps


def _get_executor(reps=1, skip=()):
    """Build (once) a reusable jitted SPMD executor over the 8 cores."""
    key = ("exec", reps, tuple(sorted(skip)))
    if key in _CACHE:
        return _CACHE[key]

    import jax
    from jax.sharding import Mesh, PartitionSpec, NamedSharding
    from jax.experimental.shard_map import shard_map
    import concourse.mybir as mybir
    from concourse.bass2jax import _bass_exec_p, install_neuronx_cc_hook

    install_neuronx_cc_hook()
    nckey = ("nc", reps, tuple(sorted(skip)))
    if nckey not in _CACHE:
        _CACHE[nckey] = _build_program(reps=reps, skip=skip)
    nc = _CACHE[nckey]
    partition_name = (nc.partition_id_tensor.name
                      if nc.partition_id_tensor else None)

    in_names, out_names, out_avals, zero_outs = [], [], [], []
    for alloc in nc.m.functions[0].allocations:
        if not isinstance(alloc, mybir.MemoryLocationSet):
            continue
        name = alloc.memorylocations[0].name
        if alloc.kind == "ExternalInput":
            if name == partition_name:
                continue
            in_names.append(name)
        elif alloc.kind == "ExternalOutput":
            out_names.append(name)
            shape = tuple(alloc.tensor_shape)
            dtype = mybir.dt.np(alloc.dtype)
            out_avals.append(jax.core.ShapedArray(shape, dtype))
            zero_outs.append(np.zeros(shape, dtype))
    n_params = len(in_names)
    all_names = in_names + out_names
    if partition_name is not None:
        all_names = all_names + [partition_name]

    from concourse.bass2jax import partition_id_tensor

    def _body(*args):
        operands = list(args)
        if partition_name is not None:
            operands.append(partition_id_tensor())
        outs = _bass_exec_p.bind(
            *operands,
            out_avals=tuple(out_avals),
            in_names=tuple(all_names),
            out_names=tuple(out_names),
            lowering_input_output_aliases=(),
            sim_require_finite=True,
            sim_require_nnan=True,
            nc=nc,
        )
        return tuple(outs)

    devices = jax.devices()[:NCORES]
    mesh = Mesh(np.asarray(devices), ("core",))
    nspec = (PartitionSpec("core"),) * (n_params + len(out_names))
    sharded = jax.jit(
        shard_map(_body, mesh=mesh, in_specs=nspec,
                  out_specs=(PartitionSpec("core"),) * len(out_names),
                  check_rep=False),
        keep_unused=True,
    )
    sharding = NamedSharding(mesh, PartitionSpec("core"))
    zeros_dev = [
        jax.device_put(
            np.zeros((NCORES * z.shape[0], *z.shape[1:]), z.dtype), sharding)
        for z in zero_outs
    ]
    _CACHE[key] = (sharded, sharding, in_names, out_names, zeros_dev)
    return _CACHE[key]


def _run(contour):
    """Returns list (per core) of dicts {sc, mn} as np arrays."""
    import jax
    sharded, sharding, in_names, out_names, zeros_dev = _get_executor()
    in_maps = _host_inputs(contour)
    concat = {
        name: np.concatenate([m[name] for m in in_maps], axis=0)
        for name in in_names
    }
    if "phi_dev" not in _CACHE:
        _CACHE["phi_dev"] = jax.device_put(concat["phi"], sharding)
    ins = [
        _CACHE["phi_dev"] if name == "phi"
        else jax.device_put(concat[name], sharding)
        for name in in_names
    ]
    outs = sharded(*ins, *zeros_dev)
    res = []
    for c in range(NCORES):
        d = {}
        for i, name in enumerate(out_names):
            arr = np.asarray(outs[i])
            rows = arr.shape[0] // NCORES
            d[name] = arr[c * rows:(c + 1) * rows]
        res.append(d)
    return res


def benchmark(contour, iters=20, reps=1, skip=()):
    """Pipelined repeated execution; returns avg seconds/iteration."""
    import time
    import jax
    sharded, sharding, in_names, out_names, zeros_dev = _get_executor(
        reps, skip)
    in_maps = _host_inputs(np.asarray(contour, dtype=np.float32))
    concat = {
        name: np.concatenate([m[name] for m in in_maps], axis=0)
        for name in in_names
    }
    ins = [jax.device_put(concat[name], sharding) for name in in_names]
    out = sharded(*ins, *zeros_dev)  # warm-up
    jax.block_until_ready(out)
    t0 = time.time()
    outs = [sharded(*ins, *zeros_dev) for _ in range(iters)]
    jax.block_until_ready(outs[-1])
    t1 = time.time()
    return (t1 - t0) / iters


def kernel(contour, *, _trace=False):
    contour = np.asarray(contour, dtype=np.float32)
    results = _run(contour)

    prod = np.zeros((NCORES, SIZE * SIZE), np.float32)
    for i in range(NCORES):
        a = i // 2
        half = i % 2
        sl = slice(half * 32768, (half + 1) * 32768)
        S = results[i]["sc"].reshape(128, NT, NPOLY)
        M = results[i]["mn"].reshape(128, NT, NPOLY)
        for pi in range(NPOLY):
            wind = np.abs(S[:, :, pi].T.ravel()) * np.float32(0.5)
            dist = np.sqrt(np.maximum(M[:, :, pi].T.ravel(), 0.0))
            prod[2 * a + pi, sl] = wind * dist
    dmap = (prod / prod.max()).astype(np.float32)
    return dmap.reshape(2, 4, SIZE, SIZE)
